# revision 19
# baseline (speedup 1.0000x reference)
"""KAN spline layer (B=16384, IN=512, OUT=1024, cubic B-splines on a uniform
grid, 8 coefficients per (in,out) pair) as a Bass/Tile kernel for 8 Trainium2
NeuronCores.

Strategy
--------
Data-parallel over the batch: each of the 8 cores gets a 2048-row shard of x
and the full (replicated) weights.

Math: for a uniform knot grid, the 8 cubic B-spline basis values per (b, i)
are: with t' = (x - g0)/h - 3 in [0.71, 4.29), cell q = floor(t') in {0..4},
u = t' - q, the four nonzero basis values are N3(u+3-m)/6 for m=0..3 placed at
basis index k = q+m.  We compute 8 "plane" tensors holding 6*B_k (the 1/6 is
folded into the weights on the host) and contract with the host-pre-transposed
weight matrix W2[k*512+i, o] = coef[i, o, k]/6 on the PE, together with the
silu/base term, accumulating y = silu(x) @ scale_base + planes @ W2 in PSUM.

Layouts: everything transposed once via PE so that in_dim lives on SBUF
partitions; spline elementwise work runs on fp16 tiles [128, 4*BCH] (constant
scalars -> tensor_scalar at 4x mode, tensor_tensor at 2x); matmuls are fp16 at
full PE rate with fp32 PSUM accumulation.
"""

import os
import numpy as np

import concourse.bass as bass
import concourse.mybir as mybir
import concourse.tile as tile
from concourse import bacc
from concourse.bass_utils import run_bass_kernel_spmd
from concourse.masks import make_identity

F32 = mybir.dt.float32
F16 = mybir.dt.float16
ALU = mybir.AluOpType
AFT = mybir.ActivationFunctionType

N_CORES = 8
B_FULL = 16384
BS = B_FULL // N_CORES          # 2048 batch rows per core
IN_DIM = 512
OUT_DIM = 1024
NK = 8                          # spline coefficients per (i, o)
NCH = IN_DIM // 128             # 4 in-dim chunks of 128 partitions
BCH = 256                       # batch columns per chunk iteration
NTB = BCH // 128                # b_tiles per batch chunk (2)


def kan_body(ctx, tc, y, x, w2, sb, tsc, tbi, bs, n_rep=1):
    nc = tc.nc
    nbch = bs // BCH
    FD = NCH * BCH              # free-dim of the elementwise spline tiles

    consts = ctx.enter_context(tc.tile_pool(name="consts", bufs=1))
    xin_pool = ctx.enter_context(tc.tile_pool(name="xin", bufs=3))
    xt_pool = ctx.enter_context(tc.tile_pool(name="xt", bufs=2))
    silu_pool = ctx.enter_context(tc.tile_pool(name="silu", bufs=2))
    plane_pool = ctx.enter_context(tc.tile_pool(name="planes", bufs=2))
    tmp_pool = ctx.enter_context(tc.tile_pool(name="tmps", bufs=1))
    yout_pool = ctx.enter_context(tc.tile_pool(name="yout", bufs=3))
    tpsum = ctx.enter_context(tc.tile_pool(name="tpsum", bufs=2, space="PSUM"))
    ypsum = ctx.enter_context(tc.tile_pool(name="ypsum", bufs=3, space="PSUM"))

    # ---- resident constants -------------------------------------------------
    w2_sb = consts.tile([128, NK * NCH, OUT_DIM], F16)
    for j in range(NK * NCH):
        nc.sync.dma_start(w2_sb[:, j, :], w2[j * 128:(j + 1) * 128, :])
    sb_sb = consts.tile([128, NCH, OUT_DIM], F16)
    for c in range(NCH):
        nc.sync.dma_start(sb_sb[:, c, :], sb[c * 128:(c + 1) * 128, :])
    tsc_sb = consts.tile([128, NCH], F32)
    tbi_sb = consts.tile([128, NCH], F32)
    for c in range(NCH):
        nc.sync.dma_start(tsc_sb[:, c:c + 1], tsc[c * 128:(c + 1) * 128, :])
        nc.sync.dma_start(tbi_sb[:, c:c + 1], tbi[c * 128:(c + 1) * 128, :])
    ident = consts.tile([128, 128], F32)
    make_identity(nc, ident)
    bias1 = consts.tile([128, 1], F32)
    nc.vector.memset(bias1, 1.0)
    bias2 = consts.tile([128, 1], F32)
    nc.vector.memset(bias2, 2.0)
    biasK = consts.tile([128, NK], F32)
    for k in range(NK):
        nc.vector.memset(biasK[:, k:k + 1], float(1 - k))

    for bc in range(nbch * n_rep):
        bc = bc % nbch
        b0 = bc * BCH

        # ---- transpose x for this batch chunk: xt[i_in_chunk, c, b] fp16 ----
        xt = xt_pool.tile([128, NCH, BCH], F16)
        tps = [tpsum.tile([128, NTB, 128], F32, tag=f"tps{c}",
                          name=f"tps{c}", bufs=1)
               for c in range(NCH)]
        for t in range(NTB):
            xin = xin_pool.tile([128, IN_DIM], F32)
            nc.sync.dma_start(xin, x[b0 + t * 128: b0 + (t + 1) * 128, :])
            for c in range(NCH):
                nc.tensor.transpose(tps[c][:, t, :],
                                    xin[:, c * 128:(c + 1) * 128], ident)
        for c in range(NCH):
            # fp32 PSUM -> fp16 SBUF cast while draining
            nc.scalar.copy(xt[:, c, :], tps[c].rearrange("p t b -> p (t b)"))

        # ---- silu(x) = x * sigmoid(x) (base term lhsT) ---------------------
        sg = silu_pool.tile([128, NCH, BCH], F16, tag="sg", name="sg")
        nc.scalar.activation(sg.rearrange("p c b -> p (c b)"),
                             xt.rearrange("p c b -> p (c b)"), AFT.Sigmoid)
        silu = silu_pool.tile([128, NCH, BCH], F16, tag="silu", name="silu")
        nc.vector.tensor_tensor(silu.rearrange("p c b -> p (c b)"),
                                xt.rearrange("p c b -> p (c b)"),
                                sg.rearrange("p c b -> p (c b)"), ALU.mult)

        # ---- spline basis planes: 6*N3(s) = relu(2-d)^3 - 4*relu(1-d)^3 ----
        # with s = t_global - k and d = |s - 2| = |t' + 1 - k|; no cell
        # indices or masks needed (the closed form is exact on all 4 pieces
        # and 0 outside the support).
        def tmp(name, k=None):
            if k is None:
                return tmp_pool.tile([128, FD], F16, tag=name, name=name)
            return tmp_pool.tile([128, FD], F16, tag=f"{name}{k % 2}",
                                 name=f"{name}{k % 2}", bufs=2)

        tp = tmp("tp")
        for c in range(NCH):
            nc.scalar.activation(tp[:, c * BCH:(c + 1) * BCH], xt[:, c, :],
                                 AFT.Identity,
                                 bias=tbi_sb[:, c:c + 1],
                                 scale=tsc_sb[:, c:c + 1])
        planes = [plane_pool.tile([128, FD], F16, tag=f"plane{k}",
                                  name=f"plane{k}") for k in range(NK)]
        for k in range(NK):
            d = tmp("d", k)
            nc.scalar.activation(d, tp, AFT.Abs, bias=biasK[:, k:k + 1],
                                 scale=1.0)
            a = tmp("a", k)
            nc.scalar.activation(a, d, AFT.Relu, bias=bias2, scale=-1.0)
            b = tmp("b", k)
            nc.scalar.activation(b, d, AFT.Relu, bias=bias1, scale=-1.0)
            a2 = tmp("a2", k)
            nc.vector.tensor_tensor(a2, a, a, ALU.mult)
            b2 = tmp("b2", k)
            nc.vector.tensor_tensor(b2, b, b, ALU.mult)
            a3 = tmp("a3", k)
            nc.vector.tensor_tensor(a3, a2, a, ALU.mult)
            b3 = tmp("b3", k)
            nc.vector.tensor_tensor(b3, b2, b, ALU.mult)
            nc.vector.scalar_tensor_tensor(planes[k], b3, -4.0, a3,
                                           ALU.mult, ALU.add)

        # ---- matmuls: y[b_tile, :] = silu.T @ sb + planes.T @ w2 -----------
        for t in range(NTB):
            yt = yout_pool.tile([128, OUT_DIM], F32)
            for h in range(2):
                ps = ypsum.tile([128, 512], F32)
                o0 = h * 512
                for c in range(NCH):
                    nc.tensor.matmul(ps, silu[:, c, t * 128:(t + 1) * 128],
                                     sb_sb[:, c, o0:o0 + 512],
                                     start=(c == 0), stop=False)
                for k in range(NK):
                    for c in range(NCH):
                        lhsT = planes[k][:, c * BCH + t * 128:
                                         c * BCH + (t + 1) * 128]
                        nc.tensor.matmul(ps, lhsT,
                                         w2_sb[:, k * NCH + c, o0:o0 + 512],
                                         start=False,
                                         stop=(k == NK - 1 and c == NCH - 1))
                nc.scalar.copy(yt[:, o0:o0 + 512], ps)
            nc.sync.dma_start(y[b0 + t * 128: b0 + (t + 1) * 128, :], yt)


def build_nc(bs=BS, n_rep=1):
    from contextlib import ExitStack

    nc = bacc.Bacc("TRN2", target_bir_lowering=False, debug=False)
    x = nc.dram_tensor("x", [bs, IN_DIM], F32, kind="ExternalInput").ap()
    w2 = nc.dram_tensor("w2", [NK * IN_DIM, OUT_DIM], F16,
                        kind="ExternalInput").ap()
    sb = nc.dram_tensor("sb", [IN_DIM, OUT_DIM], F16, kind="ExternalInput").ap()
    tsc = nc.dram_tensor("tsc", [IN_DIM, 1], F32, kind="ExternalInput").ap()
    tbi = nc.dram_tensor("tbi", [IN_DIM, 1], F32, kind="ExternalInput").ap()
    y = nc.dram_tensor("y", [bs, OUT_DIM], F32, kind="ExternalOutput").ap()
    with tile.TileContext(nc) as tc:
        with ExitStack() as ctx:
            kan_body(ctx, tc, y, x, w2, sb, tsc, tbi, bs, n_rep)
    nc.compile()
    return nc


def host_prep(grid, coef, scale_base):
    grid = np.asarray(grid, dtype=np.float32)
    coef = np.asarray(coef, dtype=np.float32)
    g0 = grid[:, 0]
    h = (grid[:, -1] - grid[:, 0]) / np.float32(grid.shape[1] - 1)
    tsc = (1.0 / h).astype(np.float32).reshape(-1, 1)
    tbi = (-g0 / h - 3.0).astype(np.float32).reshape(-1, 1)
    w2 = np.ascontiguousarray(
        np.transpose(coef, (2, 0, 1)).reshape(NK * IN_DIM, OUT_DIM) / 6.0
    ).astype(np.float16)
    sbv = np.ascontiguousarray(np.asarray(scale_base)).astype(np.float16)
    return w2, sbv, tsc, tbi


_NC_CACHE = {}


def get_nc(bs=BS):
    if bs not in _NC_CACHE:
        _NC_CACHE[bs] = build_nc(bs)
    return _NC_CACHE[bs]


def kernel(x, grid, coef, scale_base):
    x = np.ascontiguousarray(np.asarray(x, dtype=np.float32))
    w2, sbv, tsc, tbi = host_prep(grid, coef, scale_base)
    nc = get_nc(BS)
    in_maps = [
        {"x": x[c * BS:(c + 1) * BS], "w2": w2, "sb": sbv,
         "tsc": tsc, "tbi": tbi}
        for c in range(N_CORES)
    ]
    res = run_bass_kernel_spmd(nc, in_maps, core_ids=list(range(N_CORES)))
    return np.concatenate([res.results[c]["y"] for c in range(N_CORES)], axis=0)
